# revision 38
# baseline (speedup 1.0000x reference)
"""BioSSMMixer distributed Trainium2 kernel (8 NeuronCores).

Sharding: channel dimension D is split across the 8 cores (the SSM scan is
diagonal in D, so each core scans its own 128 channels with no cross-core
state). The final W_out projection contracts the full D, so the gate tensor
g = y_sp * silu(z) is exchanged with an AllToAll (d-shards -> t-shards) and
each core computes the output rows for its own T/8 slice. The AllToAll and
output matmul for batch row b=0 overlap the b=1 scan.

Host-side prep (not part of HW exec time): W_xd = W_xz[:, :D] @ W_dt is
folded so dt can be computed directly from h (each core only ever needs its
own 128 output channels of x/z/dt); h is pre-transposed to [D, B*T] bf16 so
the contraction axis lands on SBUF partitions without an on-device transpose.

Device layout: all per-channel tensors live as [d=128 partitions, (b,t) free]
tiles. Per (b, n) the recurrence s_t = decay_t*s_{t-1} + inp_t runs as a
single tensor_tensor_scan over the t axis. Bm/Cm rows (which vary with t but
not d) are broadcast across partitions with a one-hot-selector PE matmul.
The y accumulation over n runs on the otherwise-idle GpSimd engine.
"""

import os
import numpy as np
import ml_dtypes

B, T, D, N = 2, 1024, 1024, 16
NCORES = 8
DL = D // NCORES        # 128 channels per core
TL = T // NCORES        # 128 timesteps per core (output slice)
R = B * T               # 2048 rows, b-major: row = b*T + t
KT = D // 128           # 8 contraction tiles
H = 512                 # psum half-tile

BF16 = ml_dtypes.bfloat16

# Filled by kernel() when KERNEL_TRACE=1: exec_time_ns, trace tmpdir.
LAST = {}

_GRAPH_CACHE = {}


def _patch_act_tables():
    """Order activation tables so Exp and Ln resolve to the combined
    natural_log_exp_and_others table (otherwise the table-load pass
    ping-pongs between exp_and_others and natural_log)."""
    import concourse.hw_specs as hw_specs
    import concourse.bacc as bacc_mod
    orig = hw_specs.get_activation_tables.__wrapped__
    import functools

    @functools.cache
    def reordered(arch):
        # Table index (act_func_set_id) must stay aligned with
        # act_info.json's order — never reorder. Prune Exp/Ln from the
        # single-function tables instead so the load pass resolves both
        # to natural_log_exp_and_others.
        import concourse.mybir as mybir
        Act = mybir.ActivationFunctionType
        t = {k: set(v) for k, v in orig(arch).items()}
        if "natural_log_exp_and_others" in t:
            for k in ("exp_and_others", "exp_and_friends"):
                t.get(k, set()).discard(Act.Exp)
            t.get("natural_log", set()).discard(Act.Ln)
        return t

    hw_specs.get_activation_tables = reordered
    bacc_mod.get_activation_tables = reordered


def _build_graph():
    if "nc" in _GRAPH_CACHE:
        return _GRAPH_CACHE["nc"]

    import concourse.bacc as bacc
    import concourse.mybir as mybir
    from concourse import tile

    if os.environ.get('ACT_PATCH','1')=='1':
        _patch_act_tables()

    f32 = mybir.dt.float32
    bf16 = mybir.dt.bfloat16
    Alu = mybir.AluOpType
    Act = mybir.ActivationFunctionType

    nc = bacc.Bacc(
        "TRN2",
        target_bir_lowering=False,
        debug=False,
        enable_asserts=True,
        num_devices=NCORES,
    )

    hT_d = nc.dram_tensor("hT", [D, R], bf16, kind="ExternalInput")
    wx_d = nc.dram_tensor("wx", [D, DL], bf16, kind="ExternalInput")
    wz_d = nc.dram_tensor("wz", [D, DL], bf16, kind="ExternalInput")
    wd_d = nc.dram_tensor("wd", [D, DL], bf16, kind="ExternalInput")
    wbc_d = nc.dram_tensor("wbc", [D, 2 * N], bf16, kind="ExternalInput")
    wout_d = nc.dram_tensor("wout", [D, D], bf16, kind="ExternalInput")
    acol_d = nc.dram_tensor("acol", [DL, N], f32, kind="ExternalInput")
    bdt_d = nc.dram_tensor("bdt", [DL, 1], f32, kind="ExternalInput")
    dsk_d = nc.dram_tensor("dsk", [DL, 1], f32, kind="ExternalInput")
    nvth_d = nc.dram_tensor("nvth", [DL, 1], f32, kind="ExternalInput")
    hres_d = nc.dram_tensor("hres", [B, TL, D], f32, kind="ExternalInput")
    sel_d = nc.dram_tensor("sel", [2 * N, 2 * N * 128], bf16,
                           kind="ExternalInput")
    out_d = nc.dram_tensor("out", [B, TL, D], f32, kind="ExternalOutput")

    with tile.TileContext(nc) as tc:
        with (
            tc.tile_pool(name="const", bufs=1) as cpool,
            tc.tile_pool(name="work", bufs=1) as wpool,
            tc.tile_pool(name="sc", bufs=4) as scpool,
            tc.tile_pool(name="px", bufs=2, space="PSUM") as pxpool,
            tc.tile_pool(name="dram", bufs=1, space="DRAM") as dpool,
        ):
            # ---- constant loads -------------------------------------------
            hT = cpool.tile([128, KT, R], bf16)
            for j in range(KT):
                nc.sync.dma_start(hT[:, j, :], hT_d[j * 128:(j + 1) * 128, :])
            wx = cpool.tile([128, KT, DL], bf16)
            wz = cpool.tile([128, KT, DL], bf16)
            wd = cpool.tile([128, KT, DL], bf16)
            for t_, d_ in ((wx, wx_d), (wz, wz_d), (wd, wd_d)):
                for j in range(KT):
                    nc.sync.dma_start(t_[:, j, :], d_[j * 128:(j + 1) * 128, :])
            wbc = cpool.tile([128, KT, 2 * N], bf16)
            for j in range(KT):
                nc.sync.dma_start(wbc[:, j, :], wbc_d[j * 128:(j + 1) * 128, :])
            acol = cpool.tile([DL, N], f32)
            nc.sync.dma_start(acol[:], acol_d[:])
            bdt = cpool.tile([DL, 1], f32)
            nc.sync.dma_start(bdt[:], bdt_d[:])
            dsk = cpool.tile([DL, 1], f32)
            nc.sync.dma_start(dsk[:], dsk_d[:])
            nvth = cpool.tile([DL, 1], f32)
            nc.sync.dma_start(nvth[:], nvth_d[:])
            sel = cpool.tile([2 * N, 2 * N * 128], bf16)
            nc.sync.dma_start(sel[:], sel_d[:])
            # needed only for the output stage — keep out of the ramp
            wout = cpool.tile([128, KT, D], bf16)
            hres0 = cpool.tile([TL, D], f32)
            hres1 = cpool.tile([TL, D], f32)

            # ---- projections: xT/zT/dtT [128 d, R], BmCm [32, R] ----------
            xT = wpool.tile([128, R], f32)
            dtT = wpool.tile([128, R], f32)
            zT = wpool.tile([128, R], bf16)
            dtx = wpool.tile([128, R], bf16)
            yT = wpool.tile([128, R], f32)
            yT2 = wpool.tile([128, T], f32)
            bmcm = wpool.tile([2 * N, R], bf16)
            gT = wpool.tile([128, R], bf16)

            for bb in range(B):
                cs = slice(bb * T, (bb + 1) * T)
                px = pxpool.tile([128, T], f32, tag="pb", bufs=3)
                for hh in range(2):
                    hs = slice(bb * T + hh * H, bb * T + (hh + 1) * H)
                    for j in range(KT):
                        nc.tensor.matmul(px[:, hh * H:(hh + 1) * H],
                                         wx[:, j, :], hT[:, j, hs],
                                         start=(j == 0), stop=(j == KT - 1))
                nc.vector.tensor_copy(xT[:, cs], px[:])
                pz = pxpool.tile([128, T], f32, tag="pb", bufs=3)
                for hh in range(2):
                    hs = slice(bb * T + hh * H, bb * T + (hh + 1) * H)
                    for j in range(KT):
                        nc.tensor.matmul(pz[:, hh * H:(hh + 1) * H],
                                         wz[:, j, :], hT[:, j, hs],
                                         start=(j == 0), stop=(j == KT - 1))
                nc.vector.tensor_copy(zT[:, cs], pz[:])
                pd = pxpool.tile([128, T], f32, tag="pb", bufs=3)
                for hh in range(2):
                    hs = slice(bb * T + hh * H, bb * T + (hh + 1) * H)
                    for j in range(KT):
                        nc.tensor.matmul(pd[:, hh * H:(hh + 1) * H],
                                         wd[:, j, :], hT[:, j, hs],
                                         start=(j == 0), stop=(j == KT - 1))
                # softplus(x+b) = ln(1 + exp(x+b)); Exp and Ln share a table
                et = scpool.tile([128, T], f32, tag="et", bufs=2)
                nc.scalar.activation(et[:], pd[:], Act.Exp, bias=bdt[:, 0:1])
                nc.scalar.activation(dtT[:, cs], et[:], Act.Ln, bias=1.0)
                pm = pxpool.tile([32, T], f32, tag="pb", bufs=3)
                for hh in range(2):
                    hs = slice(bb * T + hh * H, bb * T + (hh + 1) * H)
                    for j in range(KT):
                        nc.tensor.matmul(pm[:, hh * H:(hh + 1) * H],
                                         wbc[:, j, :], hT[:, j, hs],
                                         start=(j == 0), stop=(j == KT - 1))
                nc.vector.tensor_copy(bmcm[:, cs], pm[:])
                nc.vector.tensor_mul(dtx[:, cs], dtT[:, cs], xT[:, cs])

            for j in range(KT):
                nc.sync.dma_start(wout[:, j, :], wout_d[j * 128:(j + 1) * 128, :])
            nc.sync.dma_start(hres0[:], hres_d[0])
            nc.sync.dma_start(hres1[:], hres_d[1])

            # ---- per-b: scan over (n), epilogue, AllToAll, out matmul -----
            gT_r = gT[:].rearrange("p (b t) -> p b t", b=B)
            for b in range(B):
                bs = slice(b * T, (b + 1) * T)
                for n in range(N):
                    decay = scpool.tile([128, T], bf16, tag="decay")
                    nc.scalar.activation(decay[:], dtT[:, bs], Act.Exp,
                                         scale=acol[:, n:n + 1])
                    pbm = pxpool.tile([128, T], f32, tag="pb", bufs=3)
                    for hh in range(2):
                        hs_d = slice(hh * H, (hh + 1) * H)
                        hs_s = slice(b * T + hh * H, b * T + (hh + 1) * H)
                        nc.tensor.matmul(pbm[:, hs_d],
                                         sel[:, n * 128:(n + 1) * 128],
                                         bmcm[:, hs_s], start=True, stop=True)
                    # PSUM->SBUF bf16 copies on the idle scalar engine buy
                    # the DVE 2x mode for both multiplies (690 vs 1219 ns)
                    bm = scpool.tile([128, T], bf16, tag="bm", bufs=3)
                    nc.scalar.activation(bm[:], pbm[:], Act.Copy)
                    nc.vector.tensor_mul(bm[:], dtx[:, bs], bm[:])
                    s = scpool.tile([128, T], bf16, tag="s")
                    nc.vector.tensor_tensor_scan(s[:], decay[:], bm[:], 0.0,
                                                 Alu.mult, Alu.add)
                    pcm = pxpool.tile([128, T], f32, tag="pb", bufs=3)
                    for hh in range(2):
                        hs_d = slice(hh * H, (hh + 1) * H)
                        hs_s = slice(b * T + hh * H, b * T + (hh + 1) * H)
                        nc.tensor.matmul(pcm[:, hs_d],
                                         sel[:, (N + n) * 128:(N + n + 1) * 128],
                                         bmcm[:, hs_s], start=True, stop=True)
                    cm = scpool.tile([128, T], bf16, tag="cm", bufs=2)
                    nc.scalar.activation(cm[:], pcm[:], Act.Copy)
                    tmp = scpool.tile([128, T], f32, tag="tmp", bufs=3)
                    nc.vector.tensor_mul(tmp[:], s[:], cm[:])
                    # two parallel accumulation chains halve the serial
                    # latency of the gpsimd y-reduction
                    acc = yT[:, bs] if n % 2 == 0 else yT2[:]
                    if n < 2:
                        nc.gpsimd.tensor_copy(acc, tmp[:])
                    else:
                        nc.gpsimd.tensor_add(acc, acc, tmp[:])
                nc.gpsimd.tensor_add(yT[:, bs], yT[:, bs], yT2[:])

                # ---- epilogue for this b ----------------------------------
                # y += D_skip*x ; spike = sigmoid(10y - 10vth) ; g = y*spk*silu(z)
                nc.vector.scalar_tensor_tensor(yT[:, bs], xT[:, bs],
                                               dsk[:, 0:1], yT[:, bs],
                                               Alu.mult, Alu.add)
                spk = scpool.tile([128, T], bf16, tag="spk", bufs=2)
                nc.scalar.activation(spk[:], yT[:, bs], Act.Sigmoid,
                                     scale=10.0, bias=nvth[:, 0:1])
                sgz = scpool.tile([128, T], bf16, tag="sgz", bufs=2)
                nc.scalar.activation(sgz[:], zT[:, bs], Act.Sigmoid)
                t1 = scpool.tile([128, T], bf16, tag="t1", bufs=2)
                nc.vector.tensor_mul(t1[:], spk[:], sgz[:])
                t2 = scpool.tile([128, T], f32, tag="t2", bufs=2)
                nc.vector.tensor_mul(t2[:], t1[:], yT[:, bs])
                nc.vector.tensor_mul(gT[:, bs], t2[:], zT[:, bs])

                # ---- AllToAll this b's g: d-shards -> t-shards ------------
                a2a_in = dpool.tile([NCORES, DL, TL], bf16, tag=f"a2ai{b}")
                a2a_out = dpool.tile([NCORES, DL, TL], bf16, tag=f"a2ao{b}")
                for j in range(NCORES):
                    nc.sync.dma_start(a2a_in[j],
                                      gT_r[:, b, j * TL:(j + 1) * TL])
                nc.gpsimd.collective_compute(
                    "AllToAll",
                    Alu.bypass,
                    replica_groups=[list(range(NCORES))],
                    ins=[a2a_in[:].opt()],
                    outs=[a2a_out[:].opt()],
                )
                ga = wpool.tile([128, NCORES, TL], bf16, tag=f"ga{b}")
                for j in range(NCORES):
                    nc.sync.dma_start(ga[:, j, :], a2a_out[j])

                # ---- out rows for this b: g_full @ W_out - h --------------
                hres_t = hres0 if b == 0 else hres1
                osb = wpool.tile([TL, D], f32, tag=f"osb{b}")
                for eh in range(2):
                    es = slice(eh * H, (eh + 1) * H)
                    po = pxpool.tile([128, H], f32, tag="px")
                    for j in range(NCORES):
                        nc.tensor.matmul(po[:], ga[:, j, :], wout[:, j, es],
                                         start=(j == 0), stop=(j == NCORES - 1))
                    nc.vector.tensor_sub(osb[:, es], po[:], hres_t[:, es])
                nc.sync.dma_start(out_d[b], osb[:])

    nc.compile()
    _GRAPH_CACHE["nc"] = nc
    return nc


def _install_ntff_hook_shim():
    """This image's antenv package lacks axon_hooks; recreate it with the
    ctypes NTFF hook from trn_agent_boot so trace=True yields exec_time_ns."""
    import sys
    import types
    try:
        import antenv.axon_hooks  # noqa: F401
        return
    except ImportError:
        pass
    import antenv
    mod = types.ModuleType("antenv.axon_hooks")
    _h = {"v": None}
    mod.set_axon_ntff_profile_hook = lambda hook: _h.update(v=hook)
    mod.get_axon_ntff_profile_hook = lambda: _h["v"]
    sys.modules["antenv.axon_hooks"] = mod
    antenv.axon_hooks = mod
    try:
        from trn_agent_boot.trn_boot import _ntff_profile_via_ctypes
        hook = _ntff_profile_via_ctypes("/opt/axon/libaxon_pjrt.so")
        mod.set_axon_ntff_profile_hook(hook)
    except Exception as e:  # degrade to no-trace
        print(f"ntff hook shim failed: {e}")


def kernel(hidden_states, W_xz, W_dt, b_dt, A_log, W_B, W_C, D_skip, W_out,
           v_th):
    h = np.asarray(hidden_states, np.float32)
    Wxz = np.asarray(W_xz, np.float32)
    Wdt = np.asarray(W_dt, np.float32)
    bdt = np.asarray(b_dt, np.float32)
    Alog = np.asarray(A_log, np.float32)
    WB = np.asarray(W_B, np.float32)
    WC = np.asarray(W_C, np.float32)
    Dsk = np.asarray(D_skip, np.float32)
    Wout = np.asarray(W_out, np.float32)
    vth = np.asarray(v_th, np.float32)

    hT = np.ascontiguousarray(h.transpose(2, 0, 1).reshape(D, R)).astype(BF16)
    Wxd = (Wxz[:, :D].astype(np.float64) @ Wdt.astype(np.float64)).astype(
        np.float32)
    A = -np.exp(Alog)
    wbc = np.ascontiguousarray(np.concatenate([WB, WC], axis=1)).astype(BF16)
    wout_bf = Wout.astype(BF16)
    wx_bf = np.ascontiguousarray(Wxz[:, :D]).astype(BF16)
    wz_bf = np.ascontiguousarray(Wxz[:, D:]).astype(BF16)
    wxd_bf = Wxd.astype(BF16)
    sel_np = np.zeros((2 * N, 2 * N * 128), dtype=BF16)
    for n in range(2 * N):
        sel_np[n, n * 128:(n + 1) * 128] = 1.0

    in_maps = []
    for k in range(NCORES):
        ds = slice(k * DL, (k + 1) * DL)
        ts = slice(k * TL, (k + 1) * TL)
        in_maps.append({
            "hT": hT,
            "wx": np.ascontiguousarray(wx_bf[:, ds]),
            "wz": np.ascontiguousarray(wz_bf[:, ds]),
            "wd": np.ascontiguousarray(wxd_bf[:, ds]),
            "wbc": wbc,
            "wout": wout_bf,
            "acol": np.ascontiguousarray(A[ds, :]),
            "bdt": np.ascontiguousarray(bdt[ds].reshape(DL, 1)),
            "dsk": np.ascontiguousarray(Dsk[ds].reshape(DL, 1)),
            "nvth": np.ascontiguousarray(
                (-10.0 * np.maximum(vth[ds], 0.1)).reshape(DL, 1)),
            "hres": np.ascontiguousarray(h[:, ts, :]),
            "sel": sel_np,
        })

    from concourse.bass_utils import run_bass_kernel_spmd

    nc = _build_graph()
    trace = os.environ.get("KERNEL_TRACE", "0") == "1"
    kwargs = {}
    if trace:
        _install_ntff_hook_shim()
        import tempfile
        tmpdir = tempfile.mkdtemp(prefix="biossm_trace_")
        kwargs = dict(trace=True, tmpdir=tmpdir)
        LAST["trace_dir"] = tmpdir
    res = run_bass_kernel_spmd(nc, in_maps, core_ids=list(range(NCORES)),
                               **kwargs)
    LAST["exec_time_ns"] = getattr(res, "exec_time_ns", None)
    out = np.concatenate(
        [np.asarray(res.results[i]["out"], np.float32) for i in range(NCORES)],
        axis=1)
    return out


# revision 39
# speedup vs baseline: 1.1993x; 1.1993x over previous
"""BioSSMMixer distributed Trainium2 kernel (8 NeuronCores).

Sharding: channel dimension D is split across the 8 cores (the SSM scan is
diagonal in D, so each core scans its own 128 channels with no cross-core
state). The final W_out projection contracts the full D, so the gate tensor
g = y_sp * silu(z) is exchanged with an AllToAll (d-shards -> t-shards) and
each core computes the output rows for its own T/8 slice. The AllToAll and
output matmul for batch row b=0 overlap the b=1 scan.

Host-side prep (not part of HW exec time): W_xd = W_xz[:, :D] @ W_dt is
folded so dt can be computed directly from h (each core only ever needs its
own 128 output channels of x/z/dt); h is pre-transposed to [D, B*T] bf16 so
the contraction axis lands on SBUF partitions without an on-device transpose.

Device layout: all per-channel tensors live as [d=128 partitions, (b,t) free]
tiles. Per (b, n) the recurrence s_t = decay_t*s_{t-1} + inp_t runs as a
single tensor_tensor_scan over the t axis. Bm/Cm rows (which vary with t but
not d) are broadcast across partitions with a one-hot-selector PE matmul.
The y accumulation over n runs on the otherwise-idle GpSimd engine.
"""

import os
import numpy as np
import ml_dtypes

B, T, D, N = 2, 1024, 1024, 16
NCORES = 8
DL = D // NCORES        # 128 channels per core
TL = T // NCORES        # 128 timesteps per core (output slice)
R = B * T               # 2048 rows, b-major: row = b*T + t
KT = D // 128           # 8 contraction tiles
H = 512                 # psum half-tile

BF16 = ml_dtypes.bfloat16

# Filled by kernel() when KERNEL_TRACE=1: exec_time_ns, trace tmpdir.
LAST = {}

_GRAPH_CACHE = {}


def _patch_act_tables():
    """Order activation tables so Exp and Ln resolve to the combined
    natural_log_exp_and_others table (otherwise the table-load pass
    ping-pongs between exp_and_others and natural_log)."""
    import concourse.hw_specs as hw_specs
    import concourse.bacc as bacc_mod
    orig = hw_specs.get_activation_tables.__wrapped__
    import functools

    @functools.cache
    def reordered(arch):
        # Table index (act_func_set_id) must stay aligned with
        # act_info.json's order — never reorder. Prune Exp/Ln from the
        # single-function tables instead so the load pass resolves both
        # to natural_log_exp_and_others.
        import concourse.mybir as mybir
        Act = mybir.ActivationFunctionType
        t = {k: set(v) for k, v in orig(arch).items()}
        if "natural_log_exp_and_others" in t:
            for k in ("exp_and_others", "exp_and_friends"):
                t.get(k, set()).discard(Act.Exp)
            t.get("natural_log", set()).discard(Act.Ln)
        return t

    hw_specs.get_activation_tables = reordered
    bacc_mod.get_activation_tables = reordered


def _build_graph():
    if "nc" in _GRAPH_CACHE:
        return _GRAPH_CACHE["nc"]

    import concourse.bacc as bacc
    import concourse.mybir as mybir
    from concourse import tile

    if os.environ.get('ACT_PATCH','1')=='1':
        _patch_act_tables()

    f32 = mybir.dt.float32
    bf16 = mybir.dt.bfloat16
    Alu = mybir.AluOpType
    Act = mybir.ActivationFunctionType

    nc = bacc.Bacc(
        "TRN2",
        target_bir_lowering=False,
        debug=False,
        enable_asserts=True,
        num_devices=NCORES,
    )

    hT_d = nc.dram_tensor("hT", [D, R], bf16, kind="ExternalInput")
    wx_d = nc.dram_tensor("wx", [D, DL], bf16, kind="ExternalInput")
    wz_d = nc.dram_tensor("wz", [D, DL], bf16, kind="ExternalInput")
    wd_d = nc.dram_tensor("wd", [D, DL], bf16, kind="ExternalInput")
    wbc_d = nc.dram_tensor("wbc", [D, 2 * N], bf16, kind="ExternalInput")
    wout_d = nc.dram_tensor("wout", [D, D], bf16, kind="ExternalInput")
    acol_d = nc.dram_tensor("acol", [DL, N], f32, kind="ExternalInput")
    bdt_d = nc.dram_tensor("bdt", [DL, 1], f32, kind="ExternalInput")
    dsk_d = nc.dram_tensor("dsk", [DL, 1], f32, kind="ExternalInput")
    nvth_d = nc.dram_tensor("nvth", [DL, 1], f32, kind="ExternalInput")
    hres_d = nc.dram_tensor("hres", [B, TL, D], f32, kind="ExternalInput")
    sel_d = nc.dram_tensor("sel", [2 * N, 2 * N * 128], bf16,
                           kind="ExternalInput")
    out_d = nc.dram_tensor("out", [B, TL, D], f32, kind="ExternalOutput")

    with tile.TileContext(nc) as tc:
        with (
            tc.tile_pool(name="const", bufs=1) as cpool,
            tc.tile_pool(name="work", bufs=1) as wpool,
            tc.tile_pool(name="sc", bufs=4) as scpool,
            tc.tile_pool(name="px", bufs=2, space="PSUM") as pxpool,
            tc.tile_pool(name="dram", bufs=1, space="DRAM") as dpool,
        ):
            # ---- constant loads -------------------------------------------
            hT = cpool.tile([128, KT, R], bf16)
            for j in range(KT):
                nc.sync.dma_start(hT[:, j, :], hT_d[j * 128:(j + 1) * 128, :])
            wx = cpool.tile([128, KT, DL], bf16)
            wz = cpool.tile([128, KT, DL], bf16)
            wd = cpool.tile([128, KT, DL], bf16)
            for t_, d_ in ((wx, wx_d), (wz, wz_d), (wd, wd_d)):
                for j in range(KT):
                    nc.sync.dma_start(t_[:, j, :], d_[j * 128:(j + 1) * 128, :])
            wbc = cpool.tile([128, KT, 2 * N], bf16)
            for j in range(KT):
                nc.sync.dma_start(wbc[:, j, :], wbc_d[j * 128:(j + 1) * 128, :])
            acol = cpool.tile([DL, N], f32)
            nc.sync.dma_start(acol[:], acol_d[:])
            bdt = cpool.tile([DL, 1], f32)
            nc.sync.dma_start(bdt[:], bdt_d[:])
            dsk = cpool.tile([DL, 1], f32)
            nc.sync.dma_start(dsk[:], dsk_d[:])
            nvth = cpool.tile([DL, 1], f32)
            nc.sync.dma_start(nvth[:], nvth_d[:])
            sel = cpool.tile([2 * N, 2 * N * 128], bf16)
            nc.sync.dma_start(sel[:], sel_d[:])
            # needed only for the output stage — keep out of the ramp
            wout = cpool.tile([128, KT, D], bf16)
            hres0 = cpool.tile([TL, D], f32)
            hres1 = cpool.tile([TL, D], f32)

            # ---- projections: xT/zT/dtT [128 d, R], BmCm [32, R] ----------
            xT = wpool.tile([128, R], bf16)
            dtT = wpool.tile([128, R], bf16)
            zT = wpool.tile([128, R], bf16)
            dtx = wpool.tile([128, R], bf16)
            yT = wpool.tile([128, R], bf16)
            yT2 = wpool.tile([128, T], bf16)
            bmcm = wpool.tile([2 * N, R], bf16)
            gT = wpool.tile([128, R], bf16)

            for bb in range(B):
                cs = slice(bb * T, (bb + 1) * T)
                px = pxpool.tile([128, T], f32, tag="pb", bufs=3)
                for hh in range(2):
                    hs = slice(bb * T + hh * H, bb * T + (hh + 1) * H)
                    for j in range(KT):
                        nc.tensor.matmul(px[:, hh * H:(hh + 1) * H],
                                         wx[:, j, :], hT[:, j, hs],
                                         start=(j == 0), stop=(j == KT - 1))
                nc.vector.tensor_copy(xT[:, cs], px[:])
                pz = pxpool.tile([128, T], f32, tag="pb", bufs=3)
                for hh in range(2):
                    hs = slice(bb * T + hh * H, bb * T + (hh + 1) * H)
                    for j in range(KT):
                        nc.tensor.matmul(pz[:, hh * H:(hh + 1) * H],
                                         wz[:, j, :], hT[:, j, hs],
                                         start=(j == 0), stop=(j == KT - 1))
                nc.vector.tensor_copy(zT[:, cs], pz[:])
                pd = pxpool.tile([128, T], f32, tag="pb", bufs=3)
                for hh in range(2):
                    hs = slice(bb * T + hh * H, bb * T + (hh + 1) * H)
                    for j in range(KT):
                        nc.tensor.matmul(pd[:, hh * H:(hh + 1) * H],
                                         wd[:, j, :], hT[:, j, hs],
                                         start=(j == 0), stop=(j == KT - 1))
                # softplus(x+b) = ln(1 + exp(x+b)); Exp and Ln share a table
                et = scpool.tile([128, T], f32, tag="et", bufs=2)
                nc.scalar.activation(et[:], pd[:], Act.Exp, bias=bdt[:, 0:1])
                nc.scalar.activation(dtT[:, cs], et[:], Act.Ln, bias=1.0)
                pm = pxpool.tile([32, T], f32, tag="pb", bufs=3)
                for hh in range(2):
                    hs = slice(bb * T + hh * H, bb * T + (hh + 1) * H)
                    for j in range(KT):
                        nc.tensor.matmul(pm[:, hh * H:(hh + 1) * H],
                                         wbc[:, j, :], hT[:, j, hs],
                                         start=(j == 0), stop=(j == KT - 1))
                nc.vector.tensor_copy(bmcm[:, cs], pm[:])
                nc.vector.tensor_mul(dtx[:, cs], dtT[:, cs], xT[:, cs])

            for j in range(KT):
                nc.sync.dma_start(wout[:, j, :], wout_d[j * 128:(j + 1) * 128, :])
            nc.sync.dma_start(hres0[:], hres_d[0])
            nc.sync.dma_start(hres1[:], hres_d[1])

            # ---- per-b: scan over (n), epilogue, AllToAll, out matmul -----
            gT_r = gT[:].rearrange("p (b t) -> p b t", b=B)
            for b in range(B):
                bs = slice(b * T, (b + 1) * T)
                for n in range(N):
                    decay = scpool.tile([128, T], bf16, tag="decay")
                    nc.scalar.activation(decay[:], dtT[:, bs], Act.Exp,
                                         scale=acol[:, n:n + 1])
                    pbm = pxpool.tile([128, T], f32, tag="pb", bufs=3)
                    for hh in range(2):
                        hs_d = slice(hh * H, (hh + 1) * H)
                        hs_s = slice(b * T + hh * H, b * T + (hh + 1) * H)
                        nc.tensor.matmul(pbm[:, hs_d],
                                         sel[:, n * 128:(n + 1) * 128],
                                         bmcm[:, hs_s], start=True, stop=True)
                    # inp = dtx * Bm_bc computed in place in PSUM; the
                    # scan reads data1 from PSUM at identical speed and no
                    # SBUF traffic is spent on an inp tile
                    nc.vector.tensor_mul(pbm[:], dtx[:, bs], pbm[:])
                    s = scpool.tile([128, T], bf16, tag="s")
                    nc.vector.tensor_tensor_scan(s[:], decay[:], pbm[:], 0.0,
                                                 Alu.mult, Alu.add)
                    pcm = pxpool.tile([128, T], f32, tag="pb", bufs=3)
                    for hh in range(2):
                        hs_d = slice(hh * H, (hh + 1) * H)
                        hs_s = slice(b * T + hh * H, b * T + (hh + 1) * H)
                        nc.tensor.matmul(pcm[:, hs_d],
                                         sel[:, (N + n) * 128:(N + n + 1) * 128],
                                         bmcm[:, hs_s], start=True, stop=True)
                    tmp = scpool.tile([128, T], bf16, tag="tmp", bufs=4)
                    nc.vector.tensor_mul(tmp[:], s[:], pcm[:])
                    # two parallel accumulation chains halve the serial
                    # latency of the gpsimd y-reduction
                    acc = yT[:, bs] if n % 2 == 0 else yT2[:]
                    if n < 2:
                        nc.gpsimd.tensor_copy(acc, tmp[:])
                    else:
                        nc.gpsimd.tensor_add(acc, acc, tmp[:])
                nc.gpsimd.tensor_add(yT[:, bs], yT[:, bs], yT2[:])

                # ---- epilogue for this b ----------------------------------
                # y += D_skip*x ; spike = sigmoid(10y - 10vth) ; g = y*spk*silu(z)
                nc.vector.scalar_tensor_tensor(yT[:, bs], xT[:, bs],
                                               dsk[:, 0:1], yT[:, bs],
                                               Alu.mult, Alu.add)
                spk = scpool.tile([128, T], bf16, tag="spk", bufs=2)
                nc.scalar.activation(spk[:], yT[:, bs], Act.Sigmoid,
                                     scale=10.0, bias=nvth[:, 0:1])
                sgz = scpool.tile([128, T], bf16, tag="sgz", bufs=2)
                nc.scalar.activation(sgz[:], zT[:, bs], Act.Sigmoid)
                t1 = scpool.tile([128, T], bf16, tag="t1", bufs=2)
                nc.vector.tensor_mul(t1[:], spk[:], sgz[:])
                t2 = scpool.tile([128, T], f32, tag="t2", bufs=2)
                nc.vector.tensor_mul(t2[:], t1[:], yT[:, bs])
                nc.vector.tensor_mul(gT[:, bs], t2[:], zT[:, bs])

                # ---- AllToAll this b's g: d-shards -> t-shards ------------
                a2a_in = dpool.tile([NCORES, DL, TL], bf16, tag=f"a2ai{b}")
                a2a_out = dpool.tile([NCORES, DL, TL], bf16, tag=f"a2ao{b}")
                for j in range(NCORES):
                    nc.sync.dma_start(a2a_in[j],
                                      gT_r[:, b, j * TL:(j + 1) * TL])
                nc.gpsimd.collective_compute(
                    "AllToAll",
                    Alu.bypass,
                    replica_groups=[list(range(NCORES))],
                    ins=[a2a_in[:].opt()],
                    outs=[a2a_out[:].opt()],
                )
                ga = wpool.tile([128, NCORES, TL], bf16, tag=f"ga{b}")
                for j in range(NCORES):
                    nc.sync.dma_start(ga[:, j, :], a2a_out[j])

                # ---- out rows for this b: g_full @ W_out - h --------------
                hres_t = hres0 if b == 0 else hres1
                osb = wpool.tile([TL, D], f32, tag=f"osb{b}")
                for eh in range(2):
                    es = slice(eh * H, (eh + 1) * H)
                    po = pxpool.tile([128, H], f32, tag="px")
                    for j in range(NCORES):
                        nc.tensor.matmul(po[:], ga[:, j, :], wout[:, j, es],
                                         start=(j == 0), stop=(j == NCORES - 1))
                    nc.vector.tensor_sub(osb[:, es], po[:], hres_t[:, es])
                nc.sync.dma_start(out_d[b], osb[:])

    nc.compile()
    _GRAPH_CACHE["nc"] = nc
    return nc


def _install_ntff_hook_shim():
    """This image's antenv package lacks axon_hooks; recreate it with the
    ctypes NTFF hook from trn_agent_boot so trace=True yields exec_time_ns."""
    import sys
    import types
    try:
        import antenv.axon_hooks  # noqa: F401
        return
    except ImportError:
        pass
    import antenv
    mod = types.ModuleType("antenv.axon_hooks")
    _h = {"v": None}
    mod.set_axon_ntff_profile_hook = lambda hook: _h.update(v=hook)
    mod.get_axon_ntff_profile_hook = lambda: _h["v"]
    sys.modules["antenv.axon_hooks"] = mod
    antenv.axon_hooks = mod
    try:
        from trn_agent_boot.trn_boot import _ntff_profile_via_ctypes
        hook = _ntff_profile_via_ctypes("/opt/axon/libaxon_pjrt.so")
        mod.set_axon_ntff_profile_hook(hook)
    except Exception as e:  # degrade to no-trace
        print(f"ntff hook shim failed: {e}")


def kernel(hidden_states, W_xz, W_dt, b_dt, A_log, W_B, W_C, D_skip, W_out,
           v_th):
    h = np.asarray(hidden_states, np.float32)
    Wxz = np.asarray(W_xz, np.float32)
    Wdt = np.asarray(W_dt, np.float32)
    bdt = np.asarray(b_dt, np.float32)
    Alog = np.asarray(A_log, np.float32)
    WB = np.asarray(W_B, np.float32)
    WC = np.asarray(W_C, np.float32)
    Dsk = np.asarray(D_skip, np.float32)
    Wout = np.asarray(W_out, np.float32)
    vth = np.asarray(v_th, np.float32)

    hT = np.ascontiguousarray(h.transpose(2, 0, 1).reshape(D, R)).astype(BF16)
    Wxd = (Wxz[:, :D].astype(np.float64) @ Wdt.astype(np.float64)).astype(
        np.float32)
    A = -np.exp(Alog)
    wbc = np.ascontiguousarray(np.concatenate([WB, WC], axis=1)).astype(BF16)
    wout_bf = Wout.astype(BF16)
    wx_bf = np.ascontiguousarray(Wxz[:, :D]).astype(BF16)
    wz_bf = np.ascontiguousarray(Wxz[:, D:]).astype(BF16)
    wxd_bf = Wxd.astype(BF16)
    sel_np = np.zeros((2 * N, 2 * N * 128), dtype=BF16)
    for n in range(2 * N):
        sel_np[n, n * 128:(n + 1) * 128] = 1.0

    in_maps = []
    for k in range(NCORES):
        ds = slice(k * DL, (k + 1) * DL)
        ts = slice(k * TL, (k + 1) * TL)
        in_maps.append({
            "hT": hT,
            "wx": np.ascontiguousarray(wx_bf[:, ds]),
            "wz": np.ascontiguousarray(wz_bf[:, ds]),
            "wd": np.ascontiguousarray(wxd_bf[:, ds]),
            "wbc": wbc,
            "wout": wout_bf,
            "acol": np.ascontiguousarray(A[ds, :]),
            "bdt": np.ascontiguousarray(bdt[ds].reshape(DL, 1)),
            "dsk": np.ascontiguousarray(Dsk[ds].reshape(DL, 1)),
            "nvth": np.ascontiguousarray(
                (-10.0 * np.maximum(vth[ds], 0.1)).reshape(DL, 1)),
            "hres": np.ascontiguousarray(h[:, ts, :]),
            "sel": sel_np,
        })

    from concourse.bass_utils import run_bass_kernel_spmd

    nc = _build_graph()
    trace = os.environ.get("KERNEL_TRACE", "0") == "1"
    kwargs = {}
    if trace:
        _install_ntff_hook_shim()
        import tempfile
        tmpdir = tempfile.mkdtemp(prefix="biossm_trace_")
        kwargs = dict(trace=True, tmpdir=tmpdir)
        LAST["trace_dir"] = tmpdir
    res = run_bass_kernel_spmd(nc, in_maps, core_ids=list(range(NCORES)),
                               **kwargs)
    LAST["exec_time_ns"] = getattr(res, "exec_time_ns", None)
    out = np.concatenate(
        [np.asarray(res.results[i]["out"], np.float32) for i in range(NCORES)],
        axis=1)
    return out
